# revision 26
# baseline (speedup 1.0000x reference)
"""Trainium2 Bass kernel for HarmonicDDSPEngine.

Strategy v5 (pure batch sharding, zero cross-core communication):
  - Each core owns 2 batches x full T as 128 partitions = 2 batches x 64
    blocks of L=2760 samples. Harmonics via the angle-split fp16 matmul:
    psum_s = W^T @ [cos;sin]_s + diag(2*lev) @ (noise-0.5)_s, noise fp8.
  - ADSR envelope*gain is piecewise-LINEAR in the sample index, so all
    blocks without a breakpoint inside are env = slope_p*iota + base_p:
    one small gpsimd iota + 4 DVE tensor_scalar quarters over partitions
    [0:120]. The <=4 blocks per batch that contain a breakpoint are
    permuted by the HOST to partitions 120..127 and their exact env rows
    are DMA'd there EARLY -- the partition ranges are disjoint, so the
    kink DMA carries no dependency on the TS writes and gates nothing.
  - sig_s = psum_s * env_s: DVE does tiles 0/2/4 straight off PSUM (1x);
    for tiles 1/3/5 ACT copies psum->fp16 and DVE multiplies at 2x.
    Per-partition abs-max via three 920-wide reduce passes on DVE.
  - Per-batch max across partitions: ONE gpsimd.partition_all_reduce over
    a masked [128, 2] input (hardware PAR ignores AP partition offsets;
    gpsimd is kept otherwise idle -- interleaving other gpsimd work
    before PAR provokes a multi-us Pool drain); batch0 owns partitions
    0:60+120:124 (col 0), batch1 owns 60:120+124:128 (col 1).
  - Normalize in quarters (DVE x3 + ACT x1), output DMA per quarter.
  - PE p-state ramp: dummy warm-up matmuls from the preamble.
"""

import os
import numpy as np

import concourse.bacc as bacc
import concourse.mybir as mybir
import concourse.tile as tile
from concourse import bass_isa
from concourse.bass_utils import run_bass_kernel_spmd

F32 = mybir.dt.float32
F16 = mybir.dt.float16
F8 = mybir.dt.float8e4
f32 = np.float32
f16 = np.float16
np_f8 = mybir.dt.np(F8)

B, T, NH = 16, 176400, 64
SR = 44100
NCORES = 8
BL = 2             # batches per core
J = 64             # t-subblocks per batch
L = 2760           # samples per subblock
TPAD = J * L       # 176640
NT = 6             # PSUM tiles per core
N = L // NT        # 460, fits one PSUM bank
Q = L // 4         # 690, iota width / env+normalize quarter
NSPEC = 4          # special (kinked) blocks per batch
NREG = J - NSPEC   # 60 regular blocks per batch

ACT_TILES = (1, 3, 5)   # sig tiles that go psum -> ACT copy -> DVE 2x

_cache = {}


def _build_nc(debug=False):
    nc = bacc.Bacc(None, num_devices=NCORES)

    tab_d = nc.dram_tensor("tab", [128, L], F16, kind="ExternalInput")
    noise_d = nc.dram_tensor("noise_p", [128, L], F8, kind="ExternalInput")
    w_d = nc.dram_tensor("wmat", [128, 128], F16, kind="ExternalInput")
    ident_d = nc.dram_tensor("identn", [128, 128], F8, kind="ExternalInput")
    consts_d = nc.dram_tensor("consts", [128, 12], F32, kind="ExternalInput")
    kink_d = nc.dram_tensor("kinks", [2 * NSPEC, L], F16,
                            kind="ExternalInput")
    out_d = nc.dram_tensor("out_sig", [128, L], F16, kind="ExternalOutput")
    if debug:
        dbg_env_d = nc.dram_tensor("dbg_env", [128, L], F16,
                                   kind="ExternalOutput")
        dbg_sig_d = nc.dram_tensor("dbg_sig", [128, L], F16,
                                   kind="ExternalOutput")
        dbg_inv_d = nc.dram_tensor("dbg_inv", [128, 1], F32,
                                   kind="ExternalOutput")

    AF = mybir.ActivationFunctionType
    OP = mybir.AluOpType
    RO = bass_isa.ReduceOp

    with tile.TileContext(nc) as tc:
        with (
            tc.tile_pool(name="const", bufs=1) as cpool,
            tc.tile_pool(name="sig", bufs=1) as spool,
            tc.tile_pool(name="small", bufs=12) as smpool,
            tc.tile_pool(name="psum", bufs=NT, space="PSUM") as ppool,
            tc.tile_pool(name="psb", bufs=1, space="PSUM") as pbpool,
        ):
            # warm-tile memset first on gpsimd so PE spins start
            # immediately; iota right behind it
            warm = smpool.tile([128, 256], F16, tag="warm")
            nc.gpsimd.memset(warm[:], 0.0)
            mx2 = smpool.tile([128, 2], F32, tag="mx2")
            tiny = smpool.tile([128, 1], F32, tag="tiny")
            nc.vector.memset(tiny[:], 0.0)
            iot = cpool.tile([128, Q], F16, tag="iot")
            nc.gpsimd.iota(iot[:], [[1, Q]], base=0, channel_multiplier=0,
                           allow_small_or_imprecise_dtypes=True)

            # ---- input DMAs, spread across issue queues ----
            tab = cpool.tile([128, L], F16, tag="tab")
            wmat = cpool.tile([128, 128], F16, tag="wmat")
            ident = cpool.tile([128, 128], F8, tag="ident")
            consts = cpool.tile([128, 12], F32, tag="consts")
            envt = cpool.tile([128, L], F16, tag="envt")
            noise_t = cpool.tile([128, L], F8, tag="noise_t")

            C2 = L // 3  # 920 = 2 psum tiles per chunk
            # SP queue, in need order; kink rows land at partitions
            # 120:128 which nothing else writes -> no dependency stall
            nc.sync.dma_start(tab[:, 0:C2], tab_d[:, 0:C2])
            nc.sync.dma_start(wmat[:], w_d[:])
            nc.sync.dma_start(ident[:], ident_d[:])
            nc.sync.dma_start(envt[120:128, :], kink_d[:])
            nc.sync.dma_start(tab[:, C2:2 * C2], tab_d[:, C2:2 * C2])
            nc.sync.dma_start(tab[:, 2 * C2:L], tab_d[:, 2 * C2:L])
            # ACT queue: small first noise chunk (gates psum0/TT0),
            # then consts (gates env TS, which has slack), then the rest
            nc.scalar.dma_start(noise_t[:, 0:N], noise_d[:, 0:N])
            nc.scalar.dma_start(consts[:], consts_d[:])
            nc.scalar.dma_start(noise_t[:, N:3 * N], noise_d[:, N:3 * N])
            nc.scalar.dma_start(noise_t[:, 3 * N:L], noise_d[:, 3 * N:L])
            # ACT-table preload rides after the DMA issues
            nc.scalar.activation(tiny[:], tiny[:], AF.Relu)

            # ---- envelope: affine per block, quarters on DVE, rows 0:120 --
            for q in range(4):
                sl = slice(q * Q, (q + 1) * Q)
                nc.vector.tensor_scalar(envt[0:120, sl], iot[0:120, :],
                                        consts[0:120, 0:1],
                                        consts[0:120, 1 + q:2 + q],
                                        OP.mult, OP.add)

            # ---- PE warm-up spins: ramp p-state + absorb DMA waits ----
            scr = pbpool.tile([128, 256], F32, tag="ps2", name="scr")
            for _ in range(10):
                nc.tensor.matmul(scr[:], warm[:, 0:128], warm[:],
                                 start=True, stop=True)

            # ---- harmonics + noise matmuls, paired to release psums early --
            psums = [ppool.tile([128, N], F32, tag="ps", name=f"ps{i}")
                     for i in range(NT)]
            for s2 in range(3):
                for s in (2 * s2, 2 * s2 + 1):
                    sl = slice(s * N, (s + 1) * N)
                    nc.tensor.matmul(psums[s][:], wmat[:], tab[:, sl],
                                     start=True, stop=False)
                for s in (2 * s2, 2 * s2 + 1):
                    sl = slice(s * N, (s + 1) * N)
                    nc.tensor.matmul(psums[s][:], ident[:], noise_t[:, sl],
                                     start=False, stop=True)

            # ---- sig = psum*env; tiles 1/3/5 via ACT copy + DVE 2x;
            #      abs-max via three 920-wide reduces on DVE ----
            sig = spool.tile([128, L], F16, tag="sig")
            harm16 = smpool.tile([128, 3 * N], F16, tag="h16")
            mxc = smpool.tile([128, 4], F16, tag="mxc")
            # reduce spans: two pair-wide, then per-tile so the last
            # reduce (critical for the fold) is short
            red_after = {1: (0, slice(0, 2 * N)), 3: (1, slice(2 * N, 4 * N)),
                         4: (2, slice(4 * N, 5 * N)), 5: (3, slice(5 * N, L))}
            for s in range(NT):
                sl = slice(s * N, (s + 1) * N)
                if s in ACT_TILES:
                    h = harm16[:, (s // 2) * N:(s // 2 + 1) * N]
                    nc.scalar.activation(h, psums[s][:], AF.Copy)
                    nc.vector.tensor_tensor(sig[:, sl], h, envt[:, sl],
                                            OP.mult)
                else:
                    nc.vector.tensor_tensor(sig[:, sl], psums[s][:],
                                            envt[:, sl], OP.mult)
                if s in red_after:
                    t, rsl = red_after[s]
                    nc.vector.tensor_reduce(
                        mxc[:, t:t + 1], sig[:, rsl],
                        axis=mybir.AxisListType.X,
                        op=OP.max, apply_absolute_value=True)

            # masked two-column per-batch fold (hardware PAR ignores AP
            # partition offsets, so feed it full-height masked columns):
            # batch0 = partitions 0:60 + 120:124 -> col 0
            # batch1 = partitions 60:120 + 124:128 -> col 1
            # engine ops must start at a partition quadrant (0/32/64/96),
            # so the batch selection uses per-partition 0/1 mask scalars
            # (consts cols 5/6) instead of partition-sliced copies
            mxp = smpool.tile([128, 1], F16, tag="mxp")
            nc.vector.tensor_reduce(mxp[:], mxc[:], axis=mybir.AxisListType.X,
                                    op=OP.max)
            # mask (cols 5/6) + eps*mask (cols 6+bl... col 7/8) in one dual
            # op each; adding eps before an all->=0 max commutes
            nc.vector.scalar_tensor_tensor(mx2[:], consts[:, 5:7],
                                           mxp[:], consts[:, 7:9],
                                           OP.mult, OP.add)
            mxr2 = smpool.tile([128, 2], F32, tag="mxr2")
            nc.gpsimd.partition_all_reduce(mxr2[:], mx2[:], 128, RO.absmax)
            inv2 = smpool.tile([128, 2], F32, tag="inv2")
            nc.vector.reciprocal(inv2[:], mxr2[:])
            invt = smpool.tile([128, 1], F32, tag="invt")
            nc.vector.tensor_scalar(invt[:], inv2[:, 0:1], consts[:, 5:6],
                                    None, OP.mult)
            inv = smpool.tile([128, 1], F32, tag="inv")
            nc.vector.scalar_tensor_tensor(inv[:], inv2[:, 1:2],
                                           consts[:, 6:7], invt[:],
                                           OP.mult, OP.add)
            if debug:
                nc.sync.dma_start(dbg_env_d[:], envt[:])
                nc.sync.dma_start(dbg_sig_d[:], sig[:])
                nc.sync.dma_start(dbg_inv_d[:], inv[:])

            # ---- normalize quarters (DVE x3 + ACT x1) and store ----
            outn = spool.tile([128, L], F16, tag="outn")
            for q in range(3):
                sl = slice(q * Q, (q + 1) * Q)
                nc.vector.tensor_scalar(outn[:, sl], sig[:, sl], inv[:],
                                        None, OP.mult)
                nc.sync.dma_start(out_d[:, sl], outn[:, sl])
            sl = slice(3 * Q, L)
            nc.scalar.activation(outn[:, sl], sig[:, sl], AF.Copy,
                                 scale=inv[:])
            nc.scalar.dma_start(out_d[:, sl], outn[:, sl])

    nc.finalize()
    return nc


def _adsr_ints(adsr):
    att_in = adsr[:, 0].astype(f32)
    dec_in = adsr[:, 1].astype(f32)
    sus = adsr[:, 2].astype(f32)
    rel_in = adsr[:, 3].astype(f32)
    a = np.floor((att_in * f32(0.5)) * f32(SR)).astype(np.int64) + 1
    d = np.floor((dec_in * f32(0.5)) * f32(SR)).astype(np.int64) + 1
    r = np.floor((rel_in * f32(0.5)) * f32(SR)).astype(np.int64) + 1
    total = a + d + r
    scale = (f32(T) / total.astype(f32)).astype(f32)
    resc = total > T
    a = np.where(resc, np.floor(a.astype(f32) * scale).astype(np.int64), a)
    d = np.where(resc, np.floor(d.astype(f32) * scale).astype(np.int64), d)
    r = np.where(resc, np.floor(r.astype(f32) * scale).astype(np.int64), r)
    s = np.maximum(T - (a + d + r), 0)
    return a, d, r, s, sus.astype(np.float64)


def _env_exact(b_idx, i, a, d, r, s, sus, g):
    """Exact envelope*gain at sample indices i (f64) for batch b_idx."""
    a_, d_, r_, s_ = (float(a[b_idx]), float(d[b_idx]),
                      float(r[b_idx]), float(s[b_idx]))
    sus_ = sus[b_idx]
    m_a = max(a_ - 1.0, 1.0)
    m_d = max(d_ - 1.0, 1.0)
    m_r = max(r_ - 1.0, 1.0)
    att = np.where(a_ > 1.0, i / m_a, 0.0)
    dec = 1.0 + (sus_ - 1.0) * (i - a_) / m_d
    rel = sus_ * (1.0 - (i - (a_ + d_ + s_)) / m_r)
    env = np.where(i < a_, att,
          np.where(i < a_ + d_, dec,
          np.where(i < a_ + d_ + s_, sus_,
          np.where(i < a_ + d_ + s_ + r_, rel, 0.0))))
    env = np.where(i < T, env, 0.0)
    return env * g[b_idx]


def _host_prep(harmonic_dist, noise_bands, adsr, gain, noise):
    """Weights in f64 (cast f16 at the end); angle split exact in f64."""
    step64 = np.float64(f32(np.float64(T / SR) / (T - 1)))
    k = np.arange(1, NH + 1, dtype=f32)
    ck64 = (f32(2.0 * np.pi * 440.0) * k).astype(np.float64)
    n = np.arange(L, dtype=np.float64)
    jj = np.arange(J, dtype=np.float64)
    phi = ck64[:, None] * (step64 * n[None, :])           # (64, L)
    theta = ck64[:, None] * (step64 * (jj[None, :] * L))  # (64, J)
    tab = np.concatenate([np.cos(phi), np.sin(phi)], axis=0).astype(f16)
    sinth, costh = np.sin(theta), np.cos(theta)           # (64, J)

    A = np.ascontiguousarray(harmonic_dist, dtype=f32).astype(np.float64)

    npad = np.zeros((B, TPAD), np_f8)
    npad[:, :T] = (noise.astype(f32) - f32(0.5)).astype(np_f8)

    lev64 = (np.mean(noise_bands.astype(f32), axis=1, dtype=f32)
             * f32(0.1)).astype(np.float64)

    a, d, r, s, sus = _adsr_ints(np.asarray(adsr))
    g64 = np.asarray(gain).astype(np.float64)[:, 0]
    A2 = a + d + s

    # per-batch block split: regular (60, linear) + special (4, kinked)
    il = np.arange(L, dtype=np.float64)
    regs, specs, kink_rows = [], [], []
    for b in range(B):
        spec = {int(a[b]) // L, int(a[b] + d[b]) // L, int(A2[b]) // L,
                J - 1}
        spec = {j for j in spec if j < J}
        reg = [j for j in range(J) if j not in spec]
        spec = sorted(spec)
        while len(spec) < NSPEC:
            spec.append(reg.pop())
        spec = sorted(spec)
        reg = [j for j in range(J) if j not in spec]
        regs.append(reg)
        specs.append(spec)
        kink_rows.append(np.stack([
            _env_exact(b, j * L + il, a, d, r, s, sus, g64)
            for j in spec]).astype(f16))

    # env slope/base for regular blocks (f64 -> f32)
    def lin_consts(b, j):
        jl = float(j * L)
        if jl + L - 1 < a[b]:
            sl_ = g64[b] / max(a[b] - 1.0, 1.0)
            bs = sl_ * jl
        elif jl >= a[b] and jl + L - 1 < a[b] + d[b]:
            m_d = max(d[b] - 1.0, 1.0)
            sl_ = g64[b] * (sus[b] - 1.0) / m_d
            bs = g64[b] * (1.0 + (sus[b] - 1.0) * (jl - a[b]) / m_d)
        elif jl >= a[b] + d[b] and jl + L - 1 < A2[b]:
            sl_, bs = 0.0, g64[b] * sus[b]
        elif jl >= A2[b]:
            m_r = max(r[b] - 1.0, 1.0)
            sl_ = -g64[b] * sus[b] / m_r
            bs = g64[b] * sus[b] * (1.0 - (jl - A2[b]) / m_r)
        else:
            raise AssertionError(f"block {j} of batch {b} spans a breakpoint")
        return sl_, bs

    # partition layout per core:
    #   0:60    batch0 regular blocks     60:120  batch1 regular blocks
    #   120:124 batch0 special blocks     124:128 batch1 special blocks
    in_maps = []
    perm_ps = []   # per batch: list of (partition, original block) pairs
    for c in range(NCORES):
        consts = np.zeros((128, 12), f32)
        noise_c = np.zeros((128, L), np_f8)
        wmat = np.zeros((128, 128), np.float64)
        sident = np.zeros((128, 128), np.float64)
        kink = np.zeros((2 * NSPEC, L), f16)
        for bl in range(BL):
            b = 2 * c + bl
            parts = (list(range(bl * NREG, (bl + 1) * NREG)) +
                     list(range(120 + bl * NSPEC, 120 + (bl + 1) * NSPEC)))
            blocks = regs[b] + specs[b]
            perm_ps.append(list(zip(parts, blocks)))
            nb = npad[b].reshape(J, L)
            for p, j in zip(parts, blocks):
                noise_c[p] = nb[j]
                wmat[:NH, p] = A[b] * sinth[:, j]
                wmat[NH:, p] = A[b] * costh[:, j]
                sident[p, p] = 2.0 * lev64[b]
            for p, j in zip(parts[:NREG], blocks[:NREG]):
                sl_, bs = lin_consts(b, j)
                consts[p, 0] = sl_
                for q in range(4):
                    consts[p, 1 + q] = bs + sl_ * (q * Q)
            for p in parts:
                consts[p, 5 + bl] = 1.0      # batch-membership masks
                consts[p, 7 + bl] = 1e-5     # eps * mask
            kink[bl * NSPEC:(bl + 1) * NSPEC] = kink_rows[b]

        in_maps.append({
            "tab": tab,
            "wmat": wmat.astype(f16),
            "identn": sident.astype(np_f8),
            "noise_p": noise_c,
            "consts": consts,
            "kinks": kink,
        })
    return in_maps, perm_ps


LAST_RESULTS = None


def kernel(base_audio, harmonic_dist, noise_bands, adsr, gain, noise):
    global LAST_RESULTS
    debug = bool(os.environ.get("BASS_DEBUG"))
    key = "nc_dbg" if debug else "nc"
    if key not in _cache:
        _cache[key] = _build_nc(debug=debug)
    nc = _cache[key]

    in_maps, perm_ps = _host_prep(
        np.asarray(harmonic_dist), np.asarray(noise_bands),
        np.asarray(adsr), np.asarray(gain), np.asarray(noise))

    trace = bool(os.environ.get("KERNEL_TRACE"))
    res = run_bass_kernel_spmd(nc, in_maps, list(range(NCORES)), trace=trace)
    LAST_RESULTS = res

    out = np.empty((B, J, L), f32)
    for c in range(NCORES):
        blk = res.results[c]["out_sig"].astype(f32)   # (128, L)
        for bl in range(BL):
            b = 2 * c + bl
            for p, j in perm_ps[b]:
                out[b, j] = blk[p]
    out = out.reshape(B, TPAD)
    return np.ascontiguousarray(out[:, :T])


# revision 27
# speedup vs baseline: 1.0172x; 1.0172x over previous
"""Trainium2 Bass kernel for HarmonicDDSPEngine.

Strategy v5 (pure batch sharding, zero cross-core communication):
  - Each core owns 2 batches x full T as 128 partitions = 2 batches x 64
    blocks of L=2760 samples. Harmonics via the angle-split fp16 matmul:
    psum_s = W^T @ [cos;sin]_s + diag(2*lev) @ (noise-0.5)_s, noise fp8.
  - ADSR envelope*gain is piecewise-LINEAR in the sample index, so all
    blocks without a breakpoint inside are env = slope_p*iota + base_p:
    one small gpsimd iota + 4 DVE tensor_scalar quarters over partitions
    [0:120]. The <=4 blocks per batch that contain a breakpoint are
    permuted by the HOST to partitions 120..127 and their exact env rows
    are DMA'd there EARLY -- the partition ranges are disjoint, so the
    kink DMA carries no dependency on the TS writes and gates nothing.
  - sig_s = psum_s * env_s: DVE does tiles 0/2/4 straight off PSUM (1x);
    for tiles 1/3/5 ACT copies psum->fp16 and DVE multiplies at 2x.
    Per-partition abs-max via three 920-wide reduce passes on DVE.
  - Per-batch max across partitions: ONE gpsimd.partition_all_reduce over
    a masked [128, 2] input (hardware PAR ignores AP partition offsets;
    gpsimd is kept otherwise idle -- interleaving other gpsimd work
    before PAR provokes a multi-us Pool drain); batch0 owns partitions
    0:60+120:124 (col 0), batch1 owns 60:120+124:128 (col 1).
  - Normalize in quarters (DVE x3 + ACT x1), output DMA per quarter.
  - PE p-state ramp: dummy warm-up matmuls from the preamble.
"""

import os
import numpy as np

import concourse.bacc as bacc
import concourse.mybir as mybir
import concourse.tile as tile
from concourse import bass_isa
from concourse.bass_utils import run_bass_kernel_spmd

F32 = mybir.dt.float32
F16 = mybir.dt.float16
F8 = mybir.dt.float8e4
f32 = np.float32
f16 = np.float16
np_f8 = mybir.dt.np(F8)

B, T, NH = 16, 176400, 64
SR = 44100
NCORES = 8
BL = 2             # batches per core
J = 64             # t-subblocks per batch
L = 2760           # samples per subblock
TPAD = J * L       # 176640
NT = 6             # PSUM tiles per core
N = L // NT        # 460, fits one PSUM bank
Q = L // 4         # 690, iota width / env+normalize quarter
NSPEC = 4          # special (kinked) blocks per batch
NREG = J - NSPEC   # 60 regular blocks per batch

ACT_TILES = (1, 3, 5)   # sig tiles that go psum -> ACT copy -> DVE 2x

_cache = {}


def _build_nc(debug=False):
    nc = bacc.Bacc(None, num_devices=NCORES)

    tab_d = nc.dram_tensor("tab", [128, L], F16, kind="ExternalInput")
    noise_d = nc.dram_tensor("noise_p", [128, L], F8, kind="ExternalInput")
    w_d = nc.dram_tensor("wmat", [128, 128], F16, kind="ExternalInput")
    ident_d = nc.dram_tensor("identn", [128, 128], F8, kind="ExternalInput")
    consts_d = nc.dram_tensor("consts", [128, 12], F32, kind="ExternalInput")
    kink_d = nc.dram_tensor("kinks", [2 * NSPEC, L], F16,
                            kind="ExternalInput")
    out_d = nc.dram_tensor("out_sig", [128, L], F16, kind="ExternalOutput")
    if debug:
        dbg_env_d = nc.dram_tensor("dbg_env", [128, L], F16,
                                   kind="ExternalOutput")
        dbg_sig_d = nc.dram_tensor("dbg_sig", [128, L], F16,
                                   kind="ExternalOutput")
        dbg_inv_d = nc.dram_tensor("dbg_inv", [128, 1], F32,
                                   kind="ExternalOutput")

    AF = mybir.ActivationFunctionType
    OP = mybir.AluOpType
    RO = bass_isa.ReduceOp

    with tile.TileContext(nc) as tc:
        with (
            tc.tile_pool(name="const", bufs=1) as cpool,
            tc.tile_pool(name="sig", bufs=1) as spool,
            tc.tile_pool(name="small", bufs=12) as smpool,
            tc.tile_pool(name="psum", bufs=NT, space="PSUM") as ppool,
            tc.tile_pool(name="psb", bufs=1, space="PSUM") as pbpool,
        ):
            # warm-tile memset first on gpsimd so PE spins start
            # immediately; iota right behind it
            warm = smpool.tile([128, 256], F16, tag="warm")
            nc.gpsimd.memset(warm[:], 0.0)
            mx2 = smpool.tile([128, 2], F32, tag="mx2")
            tiny = smpool.tile([128, 1], F32, tag="tiny")
            nc.vector.memset(tiny[:], 0.0)
            iot = cpool.tile([128, Q], F16, tag="iot")
            nc.gpsimd.iota(iot[:], [[1, Q]], base=0, channel_multiplier=0,
                           allow_small_or_imprecise_dtypes=True)

            # ---- input DMAs, spread across issue queues ----
            tab = cpool.tile([128, L], F16, tag="tab")
            wmat = cpool.tile([128, 128], F16, tag="wmat")
            ident = cpool.tile([128, 128], F8, tag="ident")
            consts = cpool.tile([128, 12], F32, tag="consts")
            envt = cpool.tile([128, L], F16, tag="envt")
            noise_t = cpool.tile([128, L], F8, tag="noise_t")

            C2 = L // 3  # 920 = 2 psum tiles per chunk
            # SP queue carries only the tab chunks so the later ones are
            # not stuck behind small transfers' issue slots
            nc.sync.dma_start(tab[:, 0:C2], tab_d[:, 0:C2])
            nc.sync.dma_start(tab[:, C2:2 * C2], tab_d[:, C2:2 * C2])
            nc.sync.dma_start(tab[:, 2 * C2:L], tab_d[:, 2 * C2:L])
            # ACT queue: everything small in need order, then the rest of
            # the noise; kink rows land at partitions 120:128 which
            # nothing else writes -> no dependency stall
            nc.scalar.dma_start(noise_t[:, 0:N], noise_d[:, 0:N])
            nc.scalar.dma_start(wmat[:], w_d[:])
            nc.scalar.dma_start(ident[:], ident_d[:])
            nc.scalar.dma_start(consts[:], consts_d[:])
            nc.scalar.dma_start(envt[120:128, :], kink_d[:])
            nc.scalar.dma_start(noise_t[:, N:3 * N], noise_d[:, N:3 * N])
            nc.scalar.dma_start(noise_t[:, 3 * N:L], noise_d[:, 3 * N:L])
            # ACT-table preload rides after the DMA issues
            nc.scalar.activation(tiny[:], tiny[:], AF.Relu)

            # ---- envelope: affine per block, quarters on DVE, rows 0:120 --
            for q in range(4):
                sl = slice(q * Q, (q + 1) * Q)
                nc.vector.tensor_scalar(envt[0:120, sl], iot[0:120, :],
                                        consts[0:120, 0:1],
                                        consts[0:120, 1 + q:2 + q],
                                        OP.mult, OP.add)

            # ---- PE warm-up spins: ramp p-state + absorb DMA waits ----
            scr = pbpool.tile([128, 256], F32, tag="ps2", name="scr")
            for _ in range(10):
                nc.tensor.matmul(scr[:], warm[:, 0:128], warm[:],
                                 start=True, stop=True)

            # ---- harmonics + noise matmuls, paired to release psums early --
            psums = [ppool.tile([128, N], F32, tag="ps", name=f"ps{i}")
                     for i in range(NT)]
            for s2 in range(3):
                for s in (2 * s2, 2 * s2 + 1):
                    sl = slice(s * N, (s + 1) * N)
                    nc.tensor.matmul(psums[s][:], wmat[:], tab[:, sl],
                                     start=True, stop=False)
                for s in (2 * s2, 2 * s2 + 1):
                    sl = slice(s * N, (s + 1) * N)
                    nc.tensor.matmul(psums[s][:], ident[:], noise_t[:, sl],
                                     start=False, stop=True)

            # ---- sig = psum*env; tiles 1/3/5 via ACT copy + DVE 2x;
            #      abs-max via three 920-wide reduces on DVE ----
            sig = spool.tile([128, L], F16, tag="sig")
            harm16 = smpool.tile([128, 3 * N], F16, tag="h16")
            mxc = smpool.tile([128, 4], F16, tag="mxc")
            # reduce spans: two pair-wide, then per-tile so the last
            # reduce (critical for the fold) is short
            red_after = {1: (0, slice(0, 2 * N)), 3: (1, slice(2 * N, 4 * N)),
                         4: (2, slice(4 * N, 5 * N)), 5: (3, slice(5 * N, L))}
            for s in range(NT):
                sl = slice(s * N, (s + 1) * N)
                if s in ACT_TILES:
                    h = harm16[:, (s // 2) * N:(s // 2 + 1) * N]
                    nc.scalar.activation(h, psums[s][:], AF.Copy)
                    nc.vector.tensor_tensor(sig[:, sl], h, envt[:, sl],
                                            OP.mult)
                else:
                    nc.vector.tensor_tensor(sig[:, sl], psums[s][:],
                                            envt[:, sl], OP.mult)
                if s in red_after:
                    t, rsl = red_after[s]
                    nc.vector.tensor_reduce(
                        mxc[:, t:t + 1], sig[:, rsl],
                        axis=mybir.AxisListType.X,
                        op=OP.max, apply_absolute_value=True)

            # masked two-column per-batch fold (hardware PAR ignores AP
            # partition offsets, so feed it full-height masked columns):
            # batch0 = partitions 0:60 + 120:124 -> col 0
            # batch1 = partitions 60:120 + 124:128 -> col 1
            # engine ops must start at a partition quadrant (0/32/64/96),
            # so the batch selection uses per-partition 0/1 mask scalars
            # (consts cols 5/6) instead of partition-sliced copies
            mxp = smpool.tile([128, 1], F16, tag="mxp")
            nc.vector.tensor_reduce(mxp[:], mxc[:], axis=mybir.AxisListType.X,
                                    op=OP.max)
            # mask (cols 5/6) + eps*mask (cols 6+bl... col 7/8) in one dual
            # op each; adding eps before an all->=0 max commutes
            nc.vector.scalar_tensor_tensor(mx2[:], consts[:, 5:7],
                                           mxp[:], consts[:, 7:9],
                                           OP.mult, OP.add)
            mxr2 = smpool.tile([128, 2], F32, tag="mxr2")
            nc.gpsimd.partition_all_reduce(mxr2[:], mx2[:], 128, RO.absmax)
            inv2 = smpool.tile([128, 2], F32, tag="inv2")
            nc.vector.reciprocal(inv2[:], mxr2[:])
            invt = smpool.tile([128, 1], F32, tag="invt")
            nc.vector.tensor_scalar(invt[:], inv2[:, 0:1], consts[:, 5:6],
                                    None, OP.mult)
            inv = smpool.tile([128, 1], F32, tag="inv")
            nc.vector.scalar_tensor_tensor(inv[:], inv2[:, 1:2],
                                           consts[:, 6:7], invt[:],
                                           OP.mult, OP.add)
            if debug:
                nc.sync.dma_start(dbg_env_d[:], envt[:])
                nc.sync.dma_start(dbg_sig_d[:], sig[:])
                nc.sync.dma_start(dbg_inv_d[:], inv[:])

            # ---- normalize quarters (DVE x3 + ACT x1) and store ----
            outn = spool.tile([128, L], F16, tag="outn")
            for q in range(3):
                sl = slice(q * Q, (q + 1) * Q)
                nc.vector.tensor_scalar(outn[:, sl], sig[:, sl], inv[:],
                                        None, OP.mult)
                nc.sync.dma_start(out_d[:, sl], outn[:, sl])
            sl = slice(3 * Q, L)
            nc.scalar.activation(outn[:, sl], sig[:, sl], AF.Copy,
                                 scale=inv[:])
            nc.scalar.dma_start(out_d[:, sl], outn[:, sl])

    nc.finalize()
    return nc


def _adsr_ints(adsr):
    att_in = adsr[:, 0].astype(f32)
    dec_in = adsr[:, 1].astype(f32)
    sus = adsr[:, 2].astype(f32)
    rel_in = adsr[:, 3].astype(f32)
    a = np.floor((att_in * f32(0.5)) * f32(SR)).astype(np.int64) + 1
    d = np.floor((dec_in * f32(0.5)) * f32(SR)).astype(np.int64) + 1
    r = np.floor((rel_in * f32(0.5)) * f32(SR)).astype(np.int64) + 1
    total = a + d + r
    scale = (f32(T) / total.astype(f32)).astype(f32)
    resc = total > T
    a = np.where(resc, np.floor(a.astype(f32) * scale).astype(np.int64), a)
    d = np.where(resc, np.floor(d.astype(f32) * scale).astype(np.int64), d)
    r = np.where(resc, np.floor(r.astype(f32) * scale).astype(np.int64), r)
    s = np.maximum(T - (a + d + r), 0)
    return a, d, r, s, sus.astype(np.float64)


def _env_exact(b_idx, i, a, d, r, s, sus, g):
    """Exact envelope*gain at sample indices i (f64) for batch b_idx."""
    a_, d_, r_, s_ = (float(a[b_idx]), float(d[b_idx]),
                      float(r[b_idx]), float(s[b_idx]))
    sus_ = sus[b_idx]
    m_a = max(a_ - 1.0, 1.0)
    m_d = max(d_ - 1.0, 1.0)
    m_r = max(r_ - 1.0, 1.0)
    att = np.where(a_ > 1.0, i / m_a, 0.0)
    dec = 1.0 + (sus_ - 1.0) * (i - a_) / m_d
    rel = sus_ * (1.0 - (i - (a_ + d_ + s_)) / m_r)
    env = np.where(i < a_, att,
          np.where(i < a_ + d_, dec,
          np.where(i < a_ + d_ + s_, sus_,
          np.where(i < a_ + d_ + s_ + r_, rel, 0.0))))
    env = np.where(i < T, env, 0.0)
    return env * g[b_idx]


def _host_prep(harmonic_dist, noise_bands, adsr, gain, noise):
    """Weights in f64 (cast f16 at the end); angle split exact in f64."""
    step64 = np.float64(f32(np.float64(T / SR) / (T - 1)))
    k = np.arange(1, NH + 1, dtype=f32)
    ck64 = (f32(2.0 * np.pi * 440.0) * k).astype(np.float64)
    n = np.arange(L, dtype=np.float64)
    jj = np.arange(J, dtype=np.float64)
    phi = ck64[:, None] * (step64 * n[None, :])           # (64, L)
    theta = ck64[:, None] * (step64 * (jj[None, :] * L))  # (64, J)
    tab = np.concatenate([np.cos(phi), np.sin(phi)], axis=0).astype(f16)
    sinth, costh = np.sin(theta), np.cos(theta)           # (64, J)

    A = np.ascontiguousarray(harmonic_dist, dtype=f32).astype(np.float64)

    npad = np.zeros((B, TPAD), np_f8)
    npad[:, :T] = (noise.astype(f32) - f32(0.5)).astype(np_f8)

    lev64 = (np.mean(noise_bands.astype(f32), axis=1, dtype=f32)
             * f32(0.1)).astype(np.float64)

    a, d, r, s, sus = _adsr_ints(np.asarray(adsr))
    g64 = np.asarray(gain).astype(np.float64)[:, 0]
    A2 = a + d + s

    # per-batch block split: regular (60, linear) + special (4, kinked)
    il = np.arange(L, dtype=np.float64)
    regs, specs, kink_rows = [], [], []
    for b in range(B):
        spec = {int(a[b]) // L, int(a[b] + d[b]) // L, int(A2[b]) // L,
                J - 1}
        spec = {j for j in spec if j < J}
        reg = [j for j in range(J) if j not in spec]
        spec = sorted(spec)
        while len(spec) < NSPEC:
            spec.append(reg.pop())
        spec = sorted(spec)
        reg = [j for j in range(J) if j not in spec]
        regs.append(reg)
        specs.append(spec)
        kink_rows.append(np.stack([
            _env_exact(b, j * L + il, a, d, r, s, sus, g64)
            for j in spec]).astype(f16))

    # env slope/base for regular blocks (f64 -> f32)
    def lin_consts(b, j):
        jl = float(j * L)
        if jl + L - 1 < a[b]:
            sl_ = g64[b] / max(a[b] - 1.0, 1.0)
            bs = sl_ * jl
        elif jl >= a[b] and jl + L - 1 < a[b] + d[b]:
            m_d = max(d[b] - 1.0, 1.0)
            sl_ = g64[b] * (sus[b] - 1.0) / m_d
            bs = g64[b] * (1.0 + (sus[b] - 1.0) * (jl - a[b]) / m_d)
        elif jl >= a[b] + d[b] and jl + L - 1 < A2[b]:
            sl_, bs = 0.0, g64[b] * sus[b]
        elif jl >= A2[b]:
            m_r = max(r[b] - 1.0, 1.0)
            sl_ = -g64[b] * sus[b] / m_r
            bs = g64[b] * sus[b] * (1.0 - (jl - A2[b]) / m_r)
        else:
            raise AssertionError(f"block {j} of batch {b} spans a breakpoint")
        return sl_, bs

    # partition layout per core:
    #   0:60    batch0 regular blocks     60:120  batch1 regular blocks
    #   120:124 batch0 special blocks     124:128 batch1 special blocks
    in_maps = []
    perm_ps = []   # per batch: list of (partition, original block) pairs
    for c in range(NCORES):
        consts = np.zeros((128, 12), f32)
        noise_c = np.zeros((128, L), np_f8)
        wmat = np.zeros((128, 128), np.float64)
        sident = np.zeros((128, 128), np.float64)
        kink = np.zeros((2 * NSPEC, L), f16)
        for bl in range(BL):
            b = 2 * c + bl
            parts = (list(range(bl * NREG, (bl + 1) * NREG)) +
                     list(range(120 + bl * NSPEC, 120 + (bl + 1) * NSPEC)))
            blocks = regs[b] + specs[b]
            perm_ps.append(list(zip(parts, blocks)))
            nb = npad[b].reshape(J, L)
            for p, j in zip(parts, blocks):
                noise_c[p] = nb[j]
                wmat[:NH, p] = A[b] * sinth[:, j]
                wmat[NH:, p] = A[b] * costh[:, j]
                sident[p, p] = 2.0 * lev64[b]
            for p, j in zip(parts[:NREG], blocks[:NREG]):
                sl_, bs = lin_consts(b, j)
                consts[p, 0] = sl_
                for q in range(4):
                    consts[p, 1 + q] = bs + sl_ * (q * Q)
            for p in parts:
                consts[p, 5 + bl] = 1.0      # batch-membership masks
                consts[p, 7 + bl] = 1e-5     # eps * mask
            kink[bl * NSPEC:(bl + 1) * NSPEC] = kink_rows[b]

        in_maps.append({
            "tab": tab,
            "wmat": wmat.astype(f16),
            "identn": sident.astype(np_f8),
            "noise_p": noise_c,
            "consts": consts,
            "kinks": kink,
        })
    return in_maps, perm_ps


LAST_RESULTS = None


def kernel(base_audio, harmonic_dist, noise_bands, adsr, gain, noise):
    global LAST_RESULTS
    debug = bool(os.environ.get("BASS_DEBUG"))
    key = "nc_dbg" if debug else "nc"
    if key not in _cache:
        _cache[key] = _build_nc(debug=debug)
    nc = _cache[key]

    in_maps, perm_ps = _host_prep(
        np.asarray(harmonic_dist), np.asarray(noise_bands),
        np.asarray(adsr), np.asarray(gain), np.asarray(noise))

    trace = bool(os.environ.get("KERNEL_TRACE"))
    res = run_bass_kernel_spmd(nc, in_maps, list(range(NCORES)), trace=trace)
    LAST_RESULTS = res

    out = np.empty((B, J, L), f32)
    for c in range(NCORES):
        blk = res.results[c]["out_sig"].astype(f32)   # (128, L)
        for bl in range(BL):
            b = 2 * c + bl
            for p, j in perm_ps[b]:
                out[b, j] = blk[p]
    out = out.reshape(B, TPAD)
    return np.ascontiguousarray(out[:, :T])
